# revision 36
# baseline (speedup 1.0000x reference)
"""CantorSetAttention Trainium2 kernel (8 NeuronCores, data-parallel).

Reference computes, for depths d=0..7, attention of every query against the
tiny Cantor index set S_d (|S_d| = 2,3,5,9,17,33,65,129; sets are nested),
then blends the 8 outputs with w = softmax(scale_weights / scale_temperature).

Fusion used here:
  A[q,j] = sum_d w_d * 1[j in S_d] * E[q,j] / Z_d(q),  E = exp(q.k_j / sqrt(D))
  rows of A sum to exactly 1 (each softmax sums to 1, sum_d w_d = 1), so with
  j* = index 0 (member of every S_d):
     out[q] = sum_{j != j*} A[q,j] * (V[j] - V[j*])  +  V[j*]
  The union minus j* is exactly 128 indices -> fits the 128-partition PE.

Device layout (per core: one batch b = core//2, query rows half = core%2):
  ST[k,q]   = K_128 @ Q^T  (8 fp16 matmuls per 512-query block, f32 PSUM)
  E = exp(ST/32)           (one ScalarE activation per block)
  Z[8,q]    = M^T E + e*   (mask matmul; e* = exp(q.k_{j*}/32) host-supplied)
  R = 1/Z                  (VectorE reciprocal, fp16)
  C[128,q]  = (w*M) R      (weighted-mask matmul)
  A = E * C                (VectorE)
  P[q,:]    = A^T-weighted (V - v*)  (fp16 AV matmuls, K=128)
Host adds v* back and upcasts to f32.

DMA notes: HWDGE/SWDGE rings stream FIFO per issuing engine well below HBM
rate, so the Q input stream is split across the ACT+DVE rings and the output
stream across the SP+GPSIMD rings, with host-side layouts arranged for
8KB-contiguous per-partition descriptor runs.
"""

import math

import numpy as np

import concourse.bass as bass
import concourse.mybir as mybir
from concourse.bass_utils import run_bass_kernel_spmd
from concourse.tile import TileContext

B, L, D = 4, 4096, 1024
NCORES = 8
ROWS_PER_CORE = (B * L) // NCORES  # 2048
N_DEPTHS = 8
INV_SQRT_D = 1.0 / math.sqrt(D)
BLK = 512  # query block per ST/E/Z/C round
NBLK = ROWS_PER_CORE // BLK  # 4
NTIL = BLK // 128  # 4
F16 = mybir.dt.float16
F32 = mybir.dt.float32

# packed [8, x] constants tensor column offsets
_M8W_OFF, _ONES_OFF, _EST_OFF = 0, 128, 256
_SMALL_COLS = _EST_OFF + ROWS_PER_CORE


def _cantor_indices(seq_len: int, depth: int) -> np.ndarray:
    pos = [0.0, 1.0]
    for _ in range(depth):
        new = []
        for i in range(len(pos) - 1):
            l, r = pos[i], pos[i + 1]
            new.append(l)
            new.append(l + (r - l) / 3.0)
        new.append(pos[-1])
        pos = new
    p32 = np.asarray(pos, dtype=np.float32)
    idx = (p32 * np.float32(seq_len - 1)).astype(np.int64)
    return np.unique(idx)


def _index_sets():
    sets = [_cantor_indices(L, d) for d in range(N_DEPTHS)]
    union = sets[-1]
    assert union[0] == 0 and len(union) == 129
    cols = union[union != 0]  # 128 non-j* indices, sorted
    member = np.zeros((N_DEPTHS, len(cols)), dtype=np.float32)
    for d, s in enumerate(sets):
        member[d] = np.isin(cols, s)
    return cols, member


_COLS, _MEMBER = _index_sets()

_NC_CACHE = None

_SPILL_SEQ = [0]


def _legalize_sync_commands(nc):
    """Walrus codegen caps sync commands (waits + updates) per ISA
    instruction at 2. Tile's vector-clock sem assignment freely attaches up
    to ~5 waits. Spill excess waits onto standalone EventSemaphore
    instructions inserted just before the offender on the same engine: the
    engine queue stalls there first, so semantics are identical."""
    for f in nc.m.functions:
        for bb in f.blocks:
            insts = bb.instructions
            idx = 0
            while idx < len(insts):
                inst = insts[idx]
                si = inst.sync_info
                if si is None:
                    idx += 1
                    continue
                waits = list(si.on_wait or [])
                updates = list(si.on_update or [])
                assert len(updates) <= 2, (inst.name, updates)
                # Drain lowers to the tiny CTRL_NO struct: one sync slot only.
                cap = 1 if isinstance(inst, mybir.InstDrain) else 2
                keep = max(0, cap - len(updates))
                if len(waits) <= keep:
                    idx += 1
                    continue
                spill, keep_waits = (
                    waits[: len(waits) - keep],
                    waits[len(waits) - keep :],
                )
                inst.sync_info = mybir.SyncInfo(on_wait=keep_waits, on_update=updates)
                pos = idx
                for i in range(0, len(spill), 2):
                    _SPILL_SEQ[0] += 1
                    ev = mybir.InstEventSemaphore(
                        name=f"WSPILL-{_SPILL_SEQ[0]}", ins=[], outs=[]
                    )
                    ev.engine = inst.engine
                    ev.sync_info = mybir.SyncInfo(
                        on_wait=spill[i : i + 2], on_update=[]
                    )
                    insts.insert(pos, ev)
                    pos += 1
                    idx += 1
                idx += 1


def _build_nc(nrep=1, mode="full"):
    # mode: "full" | "dma" (skip compute) | "compute" (skip per-rep DMAs)
    nc = bass.Bass()
    # qb[blk, p, c, q]: per-partition 8KB-contiguous block slabs
    qb = nc.declare_dram_parameter(
        "qb", [NBLK, 128, 8, BLK], F16, isOutput=False
    )
    # cpack[p, :]: kt (8*128) | vp (1024) | mt (8) packed along columns
    cpack = nc.declare_dram_parameter(
        "cpack", [128, 8 * 128 + D + N_DEPTHS], F16, isOutput=False
    )
    # small[8, :]: m8w (128) | ones8 (8, row 0) | est (2048, row 0)
    small = nc.declare_dram_parameter(
        "small", [N_DEPTHS, _SMALL_COLS], F16, isOutput=False
    )
    # out[p, tile, d]: per-partition contiguous per block; host transposes
    out = nc.declare_dram_parameter(
        "out", [128, NBLK * NTIL, D], F16, isOutput=True
    )

    with TileContext(nc) as tc:
        with (
            tc.tile_pool(name="const", bufs=1) as cpool,
            tc.tile_pool(name="qts", bufs=2) as qpool,
            tc.tile_pool(name="work", bufs=3) as wpool,
            tc.tile_pool(name="osb", bufs=3) as opool,
            tc.tile_pool(name="ps_a", bufs=3, space="PSUM") as ps_a,
            tc.tile_pool(name="ps_z", bufs=1, space="PSUM") as ps_z,
            tc.tile_pool(name="ps_o", bufs=2, space="PSUM") as ps_o,
        ):
            q_cache = {}

            def load_q(blk, engine):
                q_b = qpool.tile([128, 8, BLK], F16, tag=f"qt_{blk}")
                engine.dma_start(out=q_b, in_=qb[blk])
                q_cache[blk] = q_b
                return q_b

            # prefetch: Q split across the ACT and DVE rings (their compute
            # is emitted later so these issue immediately)
            load_q(0, nc.scalar)
            cp_t = cpool.tile([128, 8 * 128 + D + N_DEPTHS], F16, tag="cpack")
            nc.sync.dma_start(out=cp_t, in_=cpack[:])
            sm_t = cpool.tile([N_DEPTHS, _SMALL_COLS], F16, tag="small")
            nc.sync.dma_start(out=sm_t, in_=small[:])
            for b in range(1, NBLK):
                load_q(b, nc.sync if b % 2 == 1 else nc.scalar)

            kt_t = [cp_t[:, c * 128 : (c + 1) * 128] for c in range(8)]
            vp_t = cp_t[:, 1024 : 1024 + D]
            mt_t = cp_t[:, 1024 + D : 1024 + D + N_DEPTHS]
            m8w_t = sm_t[:, _M8W_OFF : _M8W_OFF + 128]
            ones8_t = sm_t[0:1, _ONES_OFF : _ONES_OFF + N_DEPTHS]
            est_t = sm_t[0:1, _EST_OFF : _EST_OFF + ROWS_PER_CORE]

            out_r = out.rearrange("p (b t) d -> p b t d", t=NTIL)

            def stageZ(rep, blk, et):
                """Z-matmuls + reciprocal: emitted before the NEXT block's ST
                so DVE's recip latency hides under that ST."""
                qs = blk * BLK
                zt = ps_z.tile([N_DEPTHS, BLK], F32, tag="zt")
                nc.tensor.matmul(zt, lhsT=mt_t, rhs=et, start=True, stop=False)
                nc.tensor.matmul(
                    zt,
                    lhsT=ones8_t,
                    rhs=est_t[:, qs : qs + BLK],
                    start=False,
                    stop=True,
                )
                rt = wpool.tile([N_DEPTHS, BLK], F16, tag="rt")
                with nc.allow_low_precision(reason="attention probs fp16"):
                    nc.vector.reciprocal(rt, zt)
                return rt

            def stage1(rep, blk):
                """ST matmuls + exp for a block."""
                if mode == "compute" or (rep == 0 and blk in q_cache):
                    q_b = q_cache[blk]
                else:
                    q_b = load_q(blk, nc.scalar if blk % 2 == 0 else nc.sync)

                st = ps_a.tile([128, BLK], F32, tag="stct")
                for c in range(8):
                    nc.tensor.matmul(
                        st,
                        lhsT=kt_t[c],
                        rhs=q_b[:, c, :],
                        start=(c == 0),
                        stop=(c == 7),
                    )
                et = wpool.tile([128, BLK], F16, tag="et")
                nc.scalar.activation(
                    et, st, mybir.ActivationFunctionType.Exp, scale=float(INV_SQRT_D)
                )
                return et

            def stage2(rep, blk, et, rt):
                """C / A / AV / output drain for a block."""
                ct = ps_a.tile([128, BLK], F32, tag="stct")
                nc.tensor.matmul(ct, lhsT=m8w_t, rhs=rt, start=True, stop=True)

                at = wpool.tile([128, BLK], F16, tag="at")
                o_blk = opool.tile([128, NTIL, D], F16, tag="osb")
                for t in range(NTIL):
                    sl = slice(t * 128, (t + 1) * 128)
                    # per-tile A = E*C so AV(t) starts as soon as slice t is up
                    nc.vector.tensor_mul(at[:, sl], et[:, sl], ct[:, sl])
                    o_ps = ps_o.tile([128, D], F32, tag="ops")
                    nc.tensor.matmul(
                        o_ps[:, 0:512],
                        lhsT=at[:, sl],
                        rhs=vp_t[:, 0:512],
                        start=True,
                        stop=True,
                    )
                    nc.tensor.matmul(
                        o_ps[:, 512:1024],
                        lhsT=at[:, sl],
                        rhs=vp_t[:, 512:1024],
                        start=True,
                        stop=True,
                    )
                    with nc.allow_low_precision(reason="fp16 output"):
                        nc.scalar.copy(o_blk[:, t, 0:640], o_ps[:, 0:640])
                        nc.vector.tensor_copy(
                            o_blk[:, t, 640:1024], o_ps[:, 640:1024]
                        )
                    if (mode != "compute" or rep == nrep - 1) and t % 2 == 1:
                        # half-block drains: mostly GPSIMD's SWDGE ring; the
                        # last block rides SP (whose input work is done) so
                        # copy-producer waits never stall the input prefetch
                        eng = nc.sync if blk == NBLK - 1 else nc.gpsimd
                        eng.dma_start(
                            out=out_r[:, blk, t - 1 : t + 1],
                            in_=o_blk[:, t - 1 : t + 1],
                        )

            if mode == "dma":
                for i in range(NBLK * nrep):
                    rep, blk = i // NBLK, i % NBLK
                    if rep == 0 and blk in q_cache:
                        q_b = q_cache[blk]
                    else:
                        q_b = load_q(blk, nc.scalar if blk % 2 == 0 else nc.sync)
                    o_blk = opool.tile([128, NTIL, D], F16, tag="osb")
                    for t in range(NTIL):
                        nc.vector.tensor_copy(
                            o_blk[:, t, 0:512], q_b[:, 0, 0:512]
                        )
                        nc.vector.tensor_copy(
                            o_blk[:, t, 512:1024], q_b[:, 1, 0:512]
                        )
                    for half in range(2):
                        eng = nc.sync if blk == NBLK - 1 else nc.gpsimd
                        eng.dma_start(
                            out=out_r[:, blk, half * 2 : half * 2 + 2],
                            in_=o_blk[:, half * 2 : half * 2 + 2],
                        )
            else:
                # software pipeline: PE order per step is
                #   Z(i-1), ST(i), C/AV(i-1)
                # so exp(i-1) hides under AV(i-2) and recip(i-1) under ST(i)
                pend = None
                for i in range(NBLK * nrep):
                    rep, blk = i // NBLK, i % NBLK
                    if pend is not None:
                        rt = stageZ(*pend)
                    et = stage1(rep, blk)
                    if pend is not None:
                        stage2(*pend, rt)
                    pend = (rep, blk, et)
                rt = stageZ(*pend)
                stage2(*pend, rt)
    _legalize_sync_commands(nc)
    return nc


def _prepare_in_maps(query, key, value, scale_weights, scale_temperature):
    sw = np.asarray(scale_weights, dtype=np.float64)[:N_DEPTHS]
    temp = float(np.asarray(scale_temperature, dtype=np.float64))
    e = np.exp(sw / temp - np.max(sw / temp))
    w = (e / e.sum()).astype(np.float32)  # [8]

    mt = _MEMBER.T.astype(np.float16)  # [128, 8]
    m8w = (_MEMBER * w[:, None]).astype(np.float16)  # [8, 128]

    in_maps = []
    vstars = []
    for core in range(NCORES):
        b, half = core // 2, core % 2
        rows = slice(half * ROWS_PER_CORE, (half + 1) * ROWS_PER_CORE)
        q = np.ascontiguousarray(query[b, rows])  # [2048, D] f32
        k_u = key[b, _COLS]  # [128, D] f32
        vstar = value[b, 0].astype(np.float32)  # [D]
        vp = (value[b, _COLS] - vstar[None, :]).astype(np.float16)
        s0 = q @ key[b, 0]  # [2048] f32
        est = np.exp(s0 * INV_SQRT_D).astype(np.float16)  # [2048]

        qt = q.T.astype(np.float16)  # [D, 2048]
        # qb[blk, p, c, q] = qt[c*128+p, blk*512+q] -> 8KB/partition slabs
        qb = np.ascontiguousarray(
            qt.reshape(8, 128, NBLK, BLK).transpose(2, 1, 0, 3)
        )
        ktp = np.ascontiguousarray(
            k_u.T.astype(np.float16).reshape(8, 128, 128).transpose(1, 0, 2)
        ).reshape(128, 1024)  # [p, c*128+j]
        cpack = np.concatenate([ktp, vp, mt], axis=1)  # [128, 2056]
        smallt = np.zeros((N_DEPTHS, _SMALL_COLS), dtype=np.float16)
        smallt[:, _M8W_OFF : _M8W_OFF + 128] = m8w
        smallt[0, _ONES_OFF : _ONES_OFF + N_DEPTHS] = 1.0
        smallt[0, _EST_OFF : _EST_OFF + ROWS_PER_CORE] = est
        in_maps.append(
            {
                "qb": qb,
                "cpack": np.ascontiguousarray(cpack),
                "small": smallt,
            }
        )
        vstars.append(vstar)
    return in_maps, vstars


def _unshard(results, vstars):
    outp = np.empty((B, L, D), dtype=np.float32)
    for core in range(NCORES):
        b, half = core // 2, core % 2
        rows = slice(half * ROWS_PER_CORE, (half + 1) * ROWS_PER_CORE)
        o = results[core]["out"]  # [128, 16, 1024] fp16
        o = o.transpose(1, 0, 2).reshape(ROWS_PER_CORE, D)
        outp[b, rows] = o.astype(np.float32) + vstars[core][None, :]
    return outp


def _run(query, key, value, t, scale_weights, scale_temperature, trace=False):
    global _NC_CACHE
    query = np.asarray(query, dtype=np.float32)
    key = np.asarray(key, dtype=np.float32)
    value = np.asarray(value, dtype=np.float32)
    assert query.shape == (B, L, D)

    in_maps, vstars = _prepare_in_maps(
        query, key, value, scale_weights, scale_temperature
    )
    if _NC_CACHE is None:
        _NC_CACHE = _build_nc()
    res = run_bass_kernel_spmd(
        _NC_CACHE, in_maps, core_ids=list(range(NCORES)), trace=trace
    )
    return _unshard(res.results, vstars), res


def kernel(query, key, value, t, scale_weights, scale_temperature):
    out, _ = _run(query, key, value, t, scale_weights, scale_temperature, trace=False)
    return out


# revision 37
# speedup vs baseline: 1.2796x; 1.2796x over previous
"""CantorSetAttention Trainium2 kernel (8 NeuronCores, data-parallel).

Reference computes, for depths d=0..7, attention of every query against the
tiny Cantor index set S_d (|S_d| = 2,3,5,9,17,33,65,129; sets are nested),
then blends the 8 outputs with w = softmax(scale_weights / scale_temperature).

Fusion used here:
  A[q,j] = sum_d w_d * 1[j in S_d] * E[q,j] / Z_d(q),  E = exp(q.k_j / sqrt(D))
  rows of A sum to exactly 1 (each softmax sums to 1, sum_d w_d = 1), so with
  j* = index 0 (member of every S_d):
     out[q] = sum_{j != j*} A[q,j] * (V[j] - V[j*])  +  V[j*]
  The union minus j* is exactly 128 indices -> fits the 128-partition PE.

Device layout (per core: one batch b = core//2, query rows half = core%2):
  ST[k,q]   = K_128 @ Q^T  (8 fp16 matmuls per 512-query block, f32 PSUM)
  E = exp(ST/32)           (one ScalarE activation per block)
  Z[8,q]    = M^T E + e*   (mask matmul; e* = exp(q.k_{j*}/32) host-supplied)
  R = 1/Z                  (VectorE reciprocal, fp16)
  C[128,q]  = (w*M) R      (weighted-mask matmul)
  A = E * C                (VectorE)
  P[q,:]    = A^T-weighted (V - v*)  (fp16 AV matmuls, K=128)
Host adds v* back and upcasts to f32.

DMA notes: HWDGE/SWDGE rings stream FIFO per issuing engine well below HBM
rate, so the Q input stream is split across the ACT+DVE rings and the output
stream across the SP+GPSIMD rings, with host-side layouts arranged for
8KB-contiguous per-partition descriptor runs.
"""

import math

import numpy as np

import concourse.bass as bass
import concourse.mybir as mybir
from concourse.bass_utils import run_bass_kernel_spmd
from concourse.tile import TileContext

B, L, D = 4, 4096, 1024
NCORES = 8
ROWS_PER_CORE = (B * L) // NCORES  # 2048
N_DEPTHS = 8
INV_SQRT_D = 1.0 / math.sqrt(D)
BLK = 512  # query block per ST/E/Z/C round
NBLK = ROWS_PER_CORE // BLK  # 4
NTIL = BLK // 128  # 4
F16 = mybir.dt.float16
F32 = mybir.dt.float32

# packed [8, x] constants tensor column offsets
_M8W_OFF, _ONES_OFF, _EST_OFF = 0, 128, 256
_SMALL_COLS = _EST_OFF + ROWS_PER_CORE


def _cantor_indices(seq_len: int, depth: int) -> np.ndarray:
    pos = [0.0, 1.0]
    for _ in range(depth):
        new = []
        for i in range(len(pos) - 1):
            l, r = pos[i], pos[i + 1]
            new.append(l)
            new.append(l + (r - l) / 3.0)
        new.append(pos[-1])
        pos = new
    p32 = np.asarray(pos, dtype=np.float32)
    idx = (p32 * np.float32(seq_len - 1)).astype(np.int64)
    return np.unique(idx)


def _index_sets():
    sets = [_cantor_indices(L, d) for d in range(N_DEPTHS)]
    union = sets[-1]
    assert union[0] == 0 and len(union) == 129
    cols = union[union != 0]  # 128 non-j* indices, sorted
    member = np.zeros((N_DEPTHS, len(cols)), dtype=np.float32)
    for d, s in enumerate(sets):
        member[d] = np.isin(cols, s)
    return cols, member


_COLS, _MEMBER = _index_sets()

_NC_CACHE = None

_SPILL_SEQ = [0]


def _legalize_sync_commands(nc):
    """Walrus codegen caps sync commands (waits + updates) per ISA
    instruction at 2. Tile's vector-clock sem assignment freely attaches up
    to ~5 waits. Spill excess waits onto standalone EventSemaphore
    instructions inserted just before the offender on the same engine: the
    engine queue stalls there first, so semantics are identical."""
    for f in nc.m.functions:
        for bb in f.blocks:
            insts = bb.instructions
            idx = 0
            while idx < len(insts):
                inst = insts[idx]
                si = inst.sync_info
                if si is None:
                    idx += 1
                    continue
                waits = list(si.on_wait or [])
                updates = list(si.on_update or [])
                assert len(updates) <= 2, (inst.name, updates)
                # Drain lowers to the tiny CTRL_NO struct: one sync slot only.
                cap = 1 if isinstance(inst, mybir.InstDrain) else 2
                keep = max(0, cap - len(updates))
                if len(waits) <= keep:
                    idx += 1
                    continue
                spill, keep_waits = (
                    waits[: len(waits) - keep],
                    waits[len(waits) - keep :],
                )
                inst.sync_info = mybir.SyncInfo(on_wait=keep_waits, on_update=updates)
                pos = idx
                for i in range(0, len(spill), 2):
                    _SPILL_SEQ[0] += 1
                    ev = mybir.InstEventSemaphore(
                        name=f"WSPILL-{_SPILL_SEQ[0]}", ins=[], outs=[]
                    )
                    ev.engine = inst.engine
                    ev.sync_info = mybir.SyncInfo(
                        on_wait=spill[i : i + 2], on_update=[]
                    )
                    insts.insert(pos, ev)
                    pos += 1
                    idx += 1
                idx += 1


def _build_nc(nrep=1, mode="full"):
    # mode: "full" | "dma" (skip compute) | "compute" (skip per-rep DMAs)
    nc = bass.Bass()
    # qb[blk, p, c, q]: per-partition 8KB-contiguous block slabs
    qb = nc.declare_dram_parameter(
        "qb", [NBLK, 128, 8, BLK], F16, isOutput=False
    )
    # cpack[p, :]: kt (8*128) | vp (1024) | mt (8) packed along columns
    cpack = nc.declare_dram_parameter(
        "cpack", [128, 8 * 128 + D + N_DEPTHS], F16, isOutput=False
    )
    # small[8, :]: m8w (128) | ones8 (8, row 0) | est (2048, row 0)
    small = nc.declare_dram_parameter(
        "small", [N_DEPTHS, _SMALL_COLS], F16, isOutput=False
    )
    # out[p, tile, d]: per-partition contiguous per block; host transposes
    out = nc.declare_dram_parameter(
        "out", [128, NBLK * NTIL, D], F16, isOutput=True
    )

    with TileContext(nc) as tc:
        with (
            tc.tile_pool(name="const", bufs=1) as cpool,
            tc.tile_pool(name="qts", bufs=2) as qpool,
            tc.tile_pool(name="work", bufs=3) as wpool,
            tc.tile_pool(name="osb", bufs=3) as opool,
            tc.tile_pool(name="ps_a", bufs=3, space="PSUM") as ps_a,
            tc.tile_pool(name="ps_z", bufs=1, space="PSUM") as ps_z,
            tc.tile_pool(name="ps_o", bufs=2, space="PSUM") as ps_o,
        ):
            q_cache = {}

            def load_q(blk, engine):
                q_b = qpool.tile([128, 8, BLK], F16, tag=f"qt_{blk}")
                engine.dma_start(out=q_b, in_=qb[blk])
                q_cache[blk] = q_b
                return q_b

            # prefetch: Q split across the ACT and DVE rings (their compute
            # is emitted later so these issue immediately)
            load_q(0, nc.scalar)
            cp_t = cpool.tile([128, 8 * 128 + D + N_DEPTHS], F16, tag="cpack")
            nc.sync.dma_start(out=cp_t, in_=cpack[:])
            sm_t = cpool.tile([N_DEPTHS, _SMALL_COLS], F16, tag="small")
            nc.sync.dma_start(out=sm_t, in_=small[:])
            for b in range(1, NBLK):
                load_q(b, nc.sync if b % 2 == 1 else nc.scalar)

            kt_t = [cp_t[:, c * 128 : (c + 1) * 128] for c in range(8)]
            vp_t = cp_t[:, 1024 : 1024 + D]
            mt_t = cp_t[:, 1024 + D : 1024 + D + N_DEPTHS]
            m8w_t = sm_t[:, _M8W_OFF : _M8W_OFF + 128]
            ones8_t = sm_t[0:1, _ONES_OFF : _ONES_OFF + N_DEPTHS]
            est_t = sm_t[0:1, _EST_OFF : _EST_OFF + ROWS_PER_CORE]

            out_r = out.rearrange("p (b t) d -> p b t d", t=NTIL)

            def stageZ(rep, blk, et):
                """Z-matmuls + reciprocal: emitted before the NEXT block's ST
                so DVE's recip latency hides under that ST."""
                qs = blk * BLK
                zt = ps_z.tile([N_DEPTHS, BLK], F32, tag="zt")
                nc.tensor.matmul(zt, lhsT=mt_t, rhs=et, start=True, stop=False)
                nc.tensor.matmul(
                    zt,
                    lhsT=ones8_t,
                    rhs=est_t[:, qs : qs + BLK],
                    start=False,
                    stop=True,
                )
                rt = wpool.tile([N_DEPTHS, BLK], F16, tag="rt")
                with nc.allow_low_precision(reason="attention probs fp16"):
                    nc.vector.reciprocal(rt, zt)
                return rt

            def stage1(rep, blk):
                """ST matmuls + exp for a block."""
                if mode == "compute" or (rep == 0 and blk in q_cache):
                    q_b = q_cache[blk]
                else:
                    q_b = load_q(blk, nc.scalar if blk % 2 == 0 else nc.sync)

                st = ps_a.tile([128, BLK], F32, tag="stct")
                for c in range(8):
                    nc.tensor.matmul(
                        st,
                        lhsT=kt_t[c],
                        rhs=q_b[:, c, :],
                        start=(c == 0),
                        stop=(c == 7),
                    )
                et = wpool.tile([128, BLK], F16, tag="et")
                nc.scalar.activation(
                    et, st, mybir.ActivationFunctionType.Exp, scale=float(INV_SQRT_D)
                )
                return et

            def stage2(rep, blk, et, rt):
                """C / A / AV / output drain for a block."""
                ct = ps_a.tile([128, BLK], F32, tag="stct")
                nc.tensor.matmul(ct, lhsT=m8w_t, rhs=rt, start=True, stop=True)

                at = wpool.tile([128, BLK], F16, tag="at")
                o_blk = opool.tile([128, NTIL, D], F16, tag="osb")
                for t in range(NTIL):
                    sl = slice(t * 128, (t + 1) * 128)
                    # per-tile A = E*C so AV(t) starts as soon as slice t is up
                    nc.vector.tensor_mul(at[:, sl], et[:, sl], ct[:, sl])
                    o_ps = ps_o.tile([128, D], F32, tag="ops")
                    nc.tensor.matmul(
                        o_ps[:, 0:512],
                        lhsT=at[:, sl],
                        rhs=vp_t[:, 0:512],
                        start=True,
                        stop=True,
                    )
                    nc.tensor.matmul(
                        o_ps[:, 512:1024],
                        lhsT=at[:, sl],
                        rhs=vp_t[:, 512:1024],
                        start=True,
                        stop=True,
                    )
                    with nc.allow_low_precision(reason="fp16 output"):
                        nc.scalar.copy(o_blk[:, t, 0:640], o_ps[:, 0:640])
                        nc.vector.tensor_copy(
                            o_blk[:, t, 640:1024], o_ps[:, 640:1024]
                        )
                    if (mode != "compute" or rep == nrep - 1) and t % 2 == 1:
                        # half-block drains: mostly GPSIMD's SWDGE ring; the
                        # last block rides SP (whose input work is done) so
                        # copy-producer waits never stall the input prefetch
                        eng = nc.sync if blk == NBLK - 1 else nc.gpsimd
                        eng.dma_start(
                            out=out_r[:, blk, t - 1 : t + 1],
                            in_=o_blk[:, t - 1 : t + 1],
                        )

            if mode == "dma":
                for i in range(NBLK * nrep):
                    rep, blk = i // NBLK, i % NBLK
                    if rep == 0 and blk in q_cache:
                        q_b = q_cache[blk]
                    else:
                        q_b = load_q(blk, nc.scalar if blk % 2 == 0 else nc.sync)
                    o_blk = opool.tile([128, NTIL, D], F16, tag="osb")
                    for t in range(NTIL):
                        nc.vector.tensor_copy(
                            o_blk[:, t, 0:512], q_b[:, 0, 0:512]
                        )
                        nc.vector.tensor_copy(
                            o_blk[:, t, 512:1024], q_b[:, 1, 0:512]
                        )
                    for half in range(2):
                        eng = nc.sync if blk == NBLK - 1 else nc.gpsimd
                        eng.dma_start(
                            out=out_r[:, blk, half * 2 : half * 2 + 2],
                            in_=o_blk[:, half * 2 : half * 2 + 2],
                        )
            else:
                # depth-2 software pipeline: PE order per step is
                #   Z(i-1), ST(i), C/AV(i-2)
                # giving every cross-engine producer (exp, recip, A-mul) a
                # full iteration of slack before its PE consumer
                work = []  # (rep, blk, et, rt)
                for i in range(NBLK * nrep):
                    rep, blk = i // NBLK, i % NBLK
                    if work and work[-1][3] is None:
                        r, b, e, _ = work[-1]
                        work[-1] = (r, b, e, stageZ(r, b, e))
                    et = stage1(rep, blk)
                    if len(work) >= 2:
                        stage2(*work.pop(0))
                    work.append((rep, blk, et, None))
                while work:
                    if work[-1][3] is None:
                        r, b, e, _ = work[-1]
                        work[-1] = (r, b, e, stageZ(r, b, e))
                    stage2(*work.pop(0))
    _legalize_sync_commands(nc)
    return nc


def _prepare_in_maps(query, key, value, scale_weights, scale_temperature):
    sw = np.asarray(scale_weights, dtype=np.float64)[:N_DEPTHS]
    temp = float(np.asarray(scale_temperature, dtype=np.float64))
    e = np.exp(sw / temp - np.max(sw / temp))
    w = (e / e.sum()).astype(np.float32)  # [8]

    mt = _MEMBER.T.astype(np.float16)  # [128, 8]
    m8w = (_MEMBER * w[:, None]).astype(np.float16)  # [8, 128]

    in_maps = []
    vstars = []
    for core in range(NCORES):
        b, half = core // 2, core % 2
        rows = slice(half * ROWS_PER_CORE, (half + 1) * ROWS_PER_CORE)
        q = np.ascontiguousarray(query[b, rows])  # [2048, D] f32
        k_u = key[b, _COLS]  # [128, D] f32
        vstar = value[b, 0].astype(np.float32)  # [D]
        vp = (value[b, _COLS] - vstar[None, :]).astype(np.float16)
        s0 = q @ key[b, 0]  # [2048] f32
        est = np.exp(s0 * INV_SQRT_D).astype(np.float16)  # [2048]

        qt = q.T.astype(np.float16)  # [D, 2048]
        # qb[blk, p, c, q] = qt[c*128+p, blk*512+q] -> 8KB/partition slabs
        qb = np.ascontiguousarray(
            qt.reshape(8, 128, NBLK, BLK).transpose(2, 1, 0, 3)
        )
        ktp = np.ascontiguousarray(
            k_u.T.astype(np.float16).reshape(8, 128, 128).transpose(1, 0, 2)
        ).reshape(128, 1024)  # [p, c*128+j]
        cpack = np.concatenate([ktp, vp, mt], axis=1)  # [128, 2056]
        smallt = np.zeros((N_DEPTHS, _SMALL_COLS), dtype=np.float16)
        smallt[:, _M8W_OFF : _M8W_OFF + 128] = m8w
        smallt[0, _ONES_OFF : _ONES_OFF + N_DEPTHS] = 1.0
        smallt[0, _EST_OFF : _EST_OFF + ROWS_PER_CORE] = est
        in_maps.append(
            {
                "qb": qb,
                "cpack": np.ascontiguousarray(cpack),
                "small": smallt,
            }
        )
        vstars.append(vstar)
    return in_maps, vstars


def _unshard(results, vstars):
    outp = np.empty((B, L, D), dtype=np.float32)
    for core in range(NCORES):
        b, half = core // 2, core % 2
        rows = slice(half * ROWS_PER_CORE, (half + 1) * ROWS_PER_CORE)
        o = results[core]["out"]  # [128, 16, 1024] fp16
        o = o.transpose(1, 0, 2).reshape(ROWS_PER_CORE, D)
        outp[b, rows] = o.astype(np.float32) + vstars[core][None, :]
    return outp


def _run(query, key, value, t, scale_weights, scale_temperature, trace=False):
    global _NC_CACHE
    query = np.asarray(query, dtype=np.float32)
    key = np.asarray(key, dtype=np.float32)
    value = np.asarray(value, dtype=np.float32)
    assert query.shape == (B, L, D)

    in_maps, vstars = _prepare_in_maps(
        query, key, value, scale_weights, scale_temperature
    )
    if _NC_CACHE is None:
        _NC_CACHE = _build_nc()
    res = run_bass_kernel_spmd(
        _NC_CACHE, in_maps, core_ids=list(range(NCORES)), trace=trace
    )
    return _unshard(res.results, vstars), res


def kernel(query, key, value, t, scale_weights, scale_temperature):
    out, _ = _run(query, key, value, t, scale_weights, scale_temperature, trace=False)
    return out


# revision 40
# speedup vs baseline: 1.3065x; 1.0210x over previous
"""CantorSetAttention Trainium2 kernel (8 NeuronCores, data-parallel).

Reference computes, for depths d=0..7, attention of every query against the
tiny Cantor index set S_d (|S_d| = 2,3,5,9,17,33,65,129; sets are nested),
then blends the 8 outputs with w = softmax(scale_weights / scale_temperature).

Fusion used here:
  A[q,j] = sum_d w_d * 1[j in S_d] * E[q,j] / Z_d(q),  E = exp(q.k_j / sqrt(D))
  rows of A sum to exactly 1 (each softmax sums to 1, sum_d w_d = 1), so with
  j* = index 0 (member of every S_d):
     out[q] = sum_{j != j*} A[q,j] * (V[j] - V[j*])  +  V[j*]
  The union minus j* is exactly 128 indices -> fits the 128-partition PE.

Device layout (per core: one batch b = core//2, query rows half = core%2):
  ST[k,q]   = K_128 @ Q^T  (8 fp16 matmuls per 512-query block, f32 PSUM)
  E = exp(ST/32)           (one ScalarE activation per block)
  Z[8,q]    = M^T E + e*   (mask matmul; e* = exp(q.k_{j*}/32) host-supplied)
  R = 1/Z                  (VectorE reciprocal, fp16)
  C[128,q]  = (w*M) R      (weighted-mask matmul)
  A = E * C                (VectorE)
  P[q,:]    = A^T-weighted (V - v*)  (fp16 AV matmuls, K=128)
Host adds v* back and upcasts to f32.

DMA notes: HWDGE/SWDGE rings stream FIFO per issuing engine well below HBM
rate, so the Q input stream is split across the ACT+DVE rings and the output
stream across the SP+GPSIMD rings, with host-side layouts arranged for
8KB-contiguous per-partition descriptor runs.
"""

import math

import numpy as np

import concourse.bass as bass
import concourse.mybir as mybir
from concourse.bass_utils import run_bass_kernel_spmd
from concourse.tile import TileContext

B, L, D = 4, 4096, 1024
NCORES = 8
ROWS_PER_CORE = (B * L) // NCORES  # 2048
N_DEPTHS = 8
INV_SQRT_D = 1.0 / math.sqrt(D)
BLK = 512  # query block per ST/E/Z/C round
NBLK = ROWS_PER_CORE // BLK  # 4
NTIL = BLK // 128  # 4
F16 = mybir.dt.float16
F32 = mybir.dt.float32

# packed [8, x] constants tensor column offsets
_M8W_OFF, _ONES_OFF, _EST_OFF = 0, 128, 256
_SMALL_COLS = _EST_OFF + ROWS_PER_CORE


def _cantor_indices(seq_len: int, depth: int) -> np.ndarray:
    pos = [0.0, 1.0]
    for _ in range(depth):
        new = []
        for i in range(len(pos) - 1):
            l, r = pos[i], pos[i + 1]
            new.append(l)
            new.append(l + (r - l) / 3.0)
        new.append(pos[-1])
        pos = new
    p32 = np.asarray(pos, dtype=np.float32)
    idx = (p32 * np.float32(seq_len - 1)).astype(np.int64)
    return np.unique(idx)


def _index_sets():
    sets = [_cantor_indices(L, d) for d in range(N_DEPTHS)]
    union = sets[-1]
    assert union[0] == 0 and len(union) == 129
    cols = union[union != 0]  # 128 non-j* indices, sorted
    member = np.zeros((N_DEPTHS, len(cols)), dtype=np.float32)
    for d, s in enumerate(sets):
        member[d] = np.isin(cols, s)
    return cols, member


_COLS, _MEMBER = _index_sets()

_NC_CACHE = None

_SPILL_SEQ = [0]


def _legalize_sync_commands(nc):
    """Walrus codegen caps sync commands (waits + updates) per ISA
    instruction at 2. Tile's vector-clock sem assignment freely attaches up
    to ~5 waits. Spill excess waits onto standalone EventSemaphore
    instructions inserted just before the offender on the same engine: the
    engine queue stalls there first, so semantics are identical."""
    for f in nc.m.functions:
        for bb in f.blocks:
            insts = bb.instructions
            idx = 0
            while idx < len(insts):
                inst = insts[idx]
                si = inst.sync_info
                if si is None:
                    idx += 1
                    continue
                waits = list(si.on_wait or [])
                updates = list(si.on_update or [])
                assert len(updates) <= 2, (inst.name, updates)
                # Drain lowers to the tiny CTRL_NO struct: one sync slot only.
                cap = 1 if isinstance(inst, mybir.InstDrain) else 2
                keep = max(0, cap - len(updates))
                if len(waits) <= keep:
                    idx += 1
                    continue
                spill, keep_waits = (
                    waits[: len(waits) - keep],
                    waits[len(waits) - keep :],
                )
                inst.sync_info = mybir.SyncInfo(on_wait=keep_waits, on_update=updates)
                pos = idx
                for i in range(0, len(spill), 2):
                    _SPILL_SEQ[0] += 1
                    ev = mybir.InstEventSemaphore(
                        name=f"WSPILL-{_SPILL_SEQ[0]}", ins=[], outs=[]
                    )
                    ev.engine = inst.engine
                    ev.sync_info = mybir.SyncInfo(
                        on_wait=spill[i : i + 2], on_update=[]
                    )
                    insts.insert(pos, ev)
                    pos += 1
                    idx += 1
                idx += 1


def _build_nc(nrep=1, mode="full", depth=3):
    # mode: "full" | "dma" (skip compute) | "compute" (skip per-rep DMAs)
    nc = bass.Bass()
    # qb[blk, p, c, q]: per-partition 8KB-contiguous block slabs
    qb = nc.declare_dram_parameter(
        "qb", [NBLK, 128, 8, BLK], F16, isOutput=False
    )
    # cpack[p, :]: kt (8*128) | vp (1024) | mt (8) packed along columns
    cpack = nc.declare_dram_parameter(
        "cpack", [128, 8 * 128 + D + N_DEPTHS], F16, isOutput=False
    )
    # small[8, :]: m8w (128) | ones8 (8, row 0) | est (2048, row 0)
    small = nc.declare_dram_parameter(
        "small", [N_DEPTHS, _SMALL_COLS], F16, isOutput=False
    )
    # out[p, tile, d]: per-partition contiguous per block; host transposes
    out = nc.declare_dram_parameter(
        "out", [128, NBLK * NTIL, D], F16, isOutput=True
    )

    with TileContext(nc) as tc:
        with (
            tc.tile_pool(name="const", bufs=1) as cpool,
            tc.tile_pool(name="qts", bufs=2) as qpool,
            tc.tile_pool(name="work", bufs=depth + 1) as wpool,
            tc.tile_pool(name="osb", bufs=3) as opool,
            tc.tile_pool(name="ps_a", bufs=3, space="PSUM") as ps_a,
            tc.tile_pool(name="ps_z", bufs=1, space="PSUM") as ps_z,
            tc.tile_pool(name="ps_o", bufs=2, space="PSUM") as ps_o,
        ):
            q_cache = {}

            def load_q(blk, engine):
                q_b = qpool.tile([128, 8, BLK], F16, tag=f"qt_{blk}")
                engine.dma_start(out=q_b, in_=qb[blk])
                q_cache[blk] = q_b
                return q_b

            # prefetch. Head-latency critical path: the first ST matmul
            # needs only kt chunk 0 + q0 chunk 0, so kt rides first on SP
            # while q0 streams per-chunk on ACT; later chunks arrive under
            # the earlier matmuls.
            cp_t = cpool.tile([128, 8 * 128 + D + N_DEPTHS], F16, tag="cpack")
            nc.sync.dma_start(out=cp_t[:, 0:1024], in_=cpack[:, 0:1024])
            q0 = qpool.tile([128, 8, BLK], F16, tag="qt_0")
            q_cache[0] = q0
            for c in range(8):
                nc.scalar.dma_start(out=q0[:, c, :], in_=qb[0, :, c, :])
            sm_t = cpool.tile([N_DEPTHS, _SMALL_COLS], F16, tag="small")
            nc.sync.dma_start(out=sm_t, in_=small[:])
            nc.sync.dma_start(out=cp_t[:, 1024:], in_=cpack[:, 1024:])
            for b in range(1, NBLK):
                load_q(b, nc.sync if b % 2 == 1 else nc.scalar)

            kt_t = [cp_t[:, c * 128 : (c + 1) * 128] for c in range(8)]
            vp_t = cp_t[:, 1024 : 1024 + D]
            mt_t = cp_t[:, 1024 + D : 1024 + D + N_DEPTHS]
            m8w_t = sm_t[:, _M8W_OFF : _M8W_OFF + 128]
            ones8_t = sm_t[0:1, _ONES_OFF : _ONES_OFF + N_DEPTHS]
            est_t = sm_t[0:1, _EST_OFF : _EST_OFF + ROWS_PER_CORE]

            out_r = out.rearrange("p (b t) d -> p b t d", t=NTIL)

            def stageZ(rep, blk, et):
                """Z-matmuls + reciprocal: emitted before the NEXT block's ST
                so DVE's recip latency hides under that ST."""
                qs = blk * BLK
                zt = ps_z.tile([N_DEPTHS, BLK], F32, tag="zt")
                nc.tensor.matmul(zt, lhsT=mt_t, rhs=et, start=True, stop=False)
                nc.tensor.matmul(
                    zt,
                    lhsT=ones8_t,
                    rhs=est_t[:, qs : qs + BLK],
                    start=False,
                    stop=True,
                )
                rt = wpool.tile([N_DEPTHS, BLK], F16, tag="rt")
                with nc.allow_low_precision(reason="attention probs fp16"):
                    nc.vector.reciprocal(rt, zt)
                return rt

            def stage1(rep, blk):
                """ST matmuls + exp for a block."""
                if mode == "compute" or (rep == 0 and blk in q_cache):
                    q_b = q_cache[blk]
                else:
                    q_b = load_q(blk, nc.scalar if blk % 2 == 0 else nc.sync)

                st = ps_a.tile([128, BLK], F32, tag="stct")
                for c in range(8):
                    nc.tensor.matmul(
                        st,
                        lhsT=kt_t[c],
                        rhs=q_b[:, c, :],
                        start=(c == 0),
                        stop=(c == 7),
                    )
                et = wpool.tile([128, BLK], F16, tag="et")
                nc.scalar.activation(
                    et, st, mybir.ActivationFunctionType.Exp, scale=float(INV_SQRT_D)
                )
                return et

            def stage2(rep, blk, et, rt):
                """C / A / AV / output drain for a block."""
                ct = ps_a.tile([128, BLK], F32, tag="stct")
                nc.tensor.matmul(ct, lhsT=m8w_t, rhs=rt, start=True, stop=True)

                at = wpool.tile([128, BLK], F16, tag="at")
                o_blk = opool.tile([128, NTIL, D], F16, tag="osb")
                for t in range(NTIL):
                    sl = slice(t * 128, (t + 1) * 128)
                    # per-tile A = E*C so AV(t) starts as soon as slice t is up
                    nc.vector.tensor_mul(at[:, sl], et[:, sl], ct[:, sl])
                    o_ps = ps_o.tile([128, D], F32, tag="ops")
                    nc.tensor.matmul(
                        o_ps[:, 0:512],
                        lhsT=at[:, sl],
                        rhs=vp_t[:, 0:512],
                        start=True,
                        stop=True,
                    )
                    nc.tensor.matmul(
                        o_ps[:, 512:1024],
                        lhsT=at[:, sl],
                        rhs=vp_t[:, 512:1024],
                        start=True,
                        stop=True,
                    )
                    with nc.allow_low_precision(reason="fp16 output"):
                        nc.scalar.copy(o_blk[:, t, 0:640], o_ps[:, 0:640])
                        nc.vector.tensor_copy(
                            o_blk[:, t, 640:1024], o_ps[:, 640:1024]
                        )
                    last = rep == nrep - 1 and blk == NBLK - 1
                    if mode == "compute" and rep != nrep - 1:
                        pass
                    elif last:
                        # final block: per-tile drains on alternating rings
                        # to minimise the tail
                        eng = nc.sync if t % 2 == 0 else nc.gpsimd
                        eng.dma_start(out=out_r[:, blk, t], in_=o_blk[:, t])
                    elif t % 2 == 1:
                        # half-block drains: GPSIMD's SWDGE ring for early
                        # blocks, SP once its input work is done, so
                        # copy-producer waits never stall the input prefetch
                        eng = nc.sync if blk == NBLK - 1 else nc.gpsimd
                        eng.dma_start(
                            out=out_r[:, blk, t - 1 : t + 1],
                            in_=o_blk[:, t - 1 : t + 1],
                        )

            if mode == "dma":
                for i in range(NBLK * nrep):
                    rep, blk = i // NBLK, i % NBLK
                    if rep == 0 and blk in q_cache:
                        q_b = q_cache[blk]
                    else:
                        q_b = load_q(blk, nc.scalar if blk % 2 == 0 else nc.sync)
                    o_blk = opool.tile([128, NTIL, D], F16, tag="osb")
                    for t in range(NTIL):
                        nc.vector.tensor_copy(
                            o_blk[:, t, 0:512], q_b[:, 0, 0:512]
                        )
                        nc.vector.tensor_copy(
                            o_blk[:, t, 512:1024], q_b[:, 1, 0:512]
                        )
                    for half in range(2):
                        eng = nc.sync if blk == NBLK - 1 else nc.gpsimd
                        eng.dma_start(
                            out=out_r[:, blk, half * 2 : half * 2 + 2],
                            in_=o_blk[:, half * 2 : half * 2 + 2],
                        )
            else:
                # software pipeline, configurable depth: PE order per step
                #   Z(i-1), ST(i), C/AV(i-depth)
                # so cross-engine producers get slack before PE consumers
                work = []  # (rep, blk, et, rt)
                for i in range(NBLK * nrep):
                    rep, blk = i // NBLK, i % NBLK
                    if work and work[-1][3] is None:
                        r, b, e, _ = work[-1]
                        work[-1] = (r, b, e, stageZ(r, b, e))
                    et = stage1(rep, blk)
                    if len(work) >= depth:
                        stage2(*work.pop(0))
                    work.append((rep, blk, et, None))
                while work:
                    if work[-1][3] is None:
                        r, b, e, _ = work[-1]
                        work[-1] = (r, b, e, stageZ(r, b, e))
                    stage2(*work.pop(0))
    _legalize_sync_commands(nc)
    return nc


def _prepare_in_maps(query, key, value, scale_weights, scale_temperature):
    sw = np.asarray(scale_weights, dtype=np.float64)[:N_DEPTHS]
    temp = float(np.asarray(scale_temperature, dtype=np.float64))
    e = np.exp(sw / temp - np.max(sw / temp))
    w = (e / e.sum()).astype(np.float32)  # [8]

    mt = _MEMBER.T.astype(np.float16)  # [128, 8]
    m8w = (_MEMBER * w[:, None]).astype(np.float16)  # [8, 128]

    in_maps = []
    vstars = []
    for core in range(NCORES):
        b, half = core // 2, core % 2
        rows = slice(half * ROWS_PER_CORE, (half + 1) * ROWS_PER_CORE)
        q = np.ascontiguousarray(query[b, rows])  # [2048, D] f32
        k_u = key[b, _COLS]  # [128, D] f32
        vstar = value[b, 0].astype(np.float32)  # [D]
        vp = (value[b, _COLS] - vstar[None, :]).astype(np.float16)
        s0 = q @ key[b, 0]  # [2048] f32
        est = np.exp(s0 * INV_SQRT_D).astype(np.float16)  # [2048]

        qt = q.T.astype(np.float16)  # [D, 2048]
        # qb[blk, p, c, q] = qt[c*128+p, blk*512+q] -> 8KB/partition slabs
        qb = np.ascontiguousarray(
            qt.reshape(8, 128, NBLK, BLK).transpose(2, 1, 0, 3)
        )
        ktp = np.ascontiguousarray(
            k_u.T.astype(np.float16).reshape(8, 128, 128).transpose(1, 0, 2)
        ).reshape(128, 1024)  # [p, c*128+j]
        cpack = np.concatenate([ktp, vp, mt], axis=1)  # [128, 2056]
        smallt = np.zeros((N_DEPTHS, _SMALL_COLS), dtype=np.float16)
        smallt[:, _M8W_OFF : _M8W_OFF + 128] = m8w
        smallt[0, _ONES_OFF : _ONES_OFF + N_DEPTHS] = 1.0
        smallt[0, _EST_OFF : _EST_OFF + ROWS_PER_CORE] = est
        in_maps.append(
            {
                "qb": qb,
                "cpack": np.ascontiguousarray(cpack),
                "small": smallt,
            }
        )
        vstars.append(vstar)
    return in_maps, vstars


def _unshard(results, vstars):
    outp = np.empty((B, L, D), dtype=np.float32)
    for core in range(NCORES):
        b, half = core // 2, core % 2
        rows = slice(half * ROWS_PER_CORE, (half + 1) * ROWS_PER_CORE)
        o = results[core]["out"]  # [128, 16, 1024] fp16
        o = o.transpose(1, 0, 2).reshape(ROWS_PER_CORE, D)
        outp[b, rows] = o.astype(np.float32) + vstars[core][None, :]
    return outp


def _run(query, key, value, t, scale_weights, scale_temperature, trace=False):
    global _NC_CACHE
    query = np.asarray(query, dtype=np.float32)
    key = np.asarray(key, dtype=np.float32)
    value = np.asarray(value, dtype=np.float32)
    assert query.shape == (B, L, D)

    in_maps, vstars = _prepare_in_maps(
        query, key, value, scale_weights, scale_temperature
    )
    if _NC_CACHE is None:
        _NC_CACHE = _build_nc()
    res = run_bass_kernel_spmd(
        _NC_CACHE, in_maps, core_ids=list(range(NCORES)), trace=trace
    )
    return _unshard(res.results, vstars), res


def kernel(query, key, value, t, scale_weights, scale_temperature):
    out, _ = _run(query, key, value, t, scale_weights, scale_temperature, trace=False)
    return out


# revision 47
# speedup vs baseline: 1.3332x; 1.0204x over previous
"""CantorSetAttention Trainium2 kernel (8 NeuronCores, data-parallel).

Reference computes, for depths d=0..7, attention of every query against the
tiny Cantor index set S_d (|S_d| = 2,3,5,9,17,33,65,129; sets are nested),
then blends the 8 outputs with w = softmax(scale_weights / scale_temperature).

Fusion used here:
  A[q,j] = sum_d w_d * 1[j in S_d] * E[q,j] / Z_d(q),  E = exp(q.k_j / sqrt(D))
  rows of A sum to exactly 1 (each softmax sums to 1, sum_d w_d = 1), so with
  j* = index 0 (member of every S_d):
     out[q] = sum_{j != j*} A[q,j] * (V[j] - V[j*])  +  V[j*]
  The union minus j* is exactly 128 indices -> fits the 128-partition PE.

Device layout (per core: one batch b = core//2, query rows half = core%2):
  ST[k,q]   = K_128 @ Q^T  (8 fp16 matmuls per 512-query block, f32 PSUM)
  E = exp(ST/32)           (one ScalarE activation per block)
  Z[8,q]    = M^T E + e*   (mask matmul; e* = exp(q.k_{j*}/32) host-supplied)
  R = 1/Z                  (VectorE reciprocal, fp16)
  C[128,q]  = (w*M) R      (weighted-mask matmul)
  A = E * C                (VectorE)
  P[q,:]    = A^T-weighted (V - v*)  (fp16 AV matmuls, K=128)
Host adds v* back and upcasts to f32.

DMA notes: HWDGE/SWDGE rings stream FIFO per issuing engine well below HBM
rate, so the Q input stream is split across the ACT+DVE rings and the output
stream across the SP+GPSIMD rings, with host-side layouts arranged for
8KB-contiguous per-partition descriptor runs.
"""

import math

import numpy as np

import concourse.bass as bass
import concourse.mybir as mybir
from concourse.bass_utils import run_bass_kernel_spmd
from concourse.tile import TileContext, add_dep_helper

B, L, D = 4, 4096, 1024
NCORES = 8
ROWS_PER_CORE = (B * L) // NCORES  # 2048
N_DEPTHS = 8
INV_SQRT_D = 1.0 / math.sqrt(D)
BLK = 512  # query block per ST/E/Z/C round
NBLK = ROWS_PER_CORE // BLK  # 4
NTIL = BLK // 128  # 4
F16 = mybir.dt.float16
F32 = mybir.dt.float32

# packed [8, x] constants tensor column offsets
_M8W_OFF, _ONES_OFF, _EST_OFF = 0, 128, 256
_SMALL_COLS = _EST_OFF + ROWS_PER_CORE


def _cantor_indices(seq_len: int, depth: int) -> np.ndarray:
    pos = [0.0, 1.0]
    for _ in range(depth):
        new = []
        for i in range(len(pos) - 1):
            l, r = pos[i], pos[i + 1]
            new.append(l)
            new.append(l + (r - l) / 3.0)
        new.append(pos[-1])
        pos = new
    p32 = np.asarray(pos, dtype=np.float32)
    idx = (p32 * np.float32(seq_len - 1)).astype(np.int64)
    return np.unique(idx)


def _index_sets():
    sets = [_cantor_indices(L, d) for d in range(N_DEPTHS)]
    union = sets[-1]
    assert union[0] == 0 and len(union) == 129
    cols = union[union != 0]  # 128 non-j* indices, sorted
    member = np.zeros((N_DEPTHS, len(cols)), dtype=np.float32)
    for d, s in enumerate(sets):
        member[d] = np.isin(cols, s)
    return cols, member


_COLS, _MEMBER = _index_sets()

_NC_CACHE = None

_SPILL_SEQ = [0]


def _dedupe_ldweights(nc):
    """Delete a standalone InstLdweights whose weights AP is identical to
    the immediately preceding PE Ldweights (the stationary is already in the
    array; consecutive AV matmuls share it). Waits migrate to the next
    instruction so the legalizer can re-cap them."""
    for f in nc.m.functions:
        for bb in f.blocks:
            insts = bb.instructions
            last_ldw_ap = None
            idx = 0
            while idx < len(insts):
                inst = insts[idx]
                if str(inst.engine) != "EngineType.PE":
                    idx += 1
                    continue
                tn = type(inst).__name__
                if tn == "InstLdweights":
                    ap = str(inst.ins[0]) if inst.ins else None
                    si = inst.sync_info
                    has_sync = si is not None and (si.on_wait or si.on_update)
                    if ap is not None and ap == last_ldw_ap and not has_sync:
                        del insts[idx]
                        continue
                    last_ldw_ap = ap
                idx += 1


def _legalize_sync_commands(nc):
    """Walrus codegen caps sync commands (waits + updates) per ISA
    instruction at 2. Tile's vector-clock sem assignment freely attaches up
    to ~5 waits. Spill excess waits onto standalone EventSemaphore
    instructions inserted just before the offender on the same engine: the
    engine queue stalls there first, so semantics are identical."""
    for f in nc.m.functions:
        for bb in f.blocks:
            insts = bb.instructions
            idx = 0
            while idx < len(insts):
                inst = insts[idx]
                si = inst.sync_info
                if si is None:
                    idx += 1
                    continue
                waits = list(si.on_wait or [])
                updates = list(si.on_update or [])
                assert len(updates) <= 2, (inst.name, updates)
                # Drain lowers to the tiny CTRL_NO struct: one sync slot only.
                cap = 1 if isinstance(inst, mybir.InstDrain) else 2
                keep = max(0, cap - len(updates))
                if len(waits) <= keep:
                    idx += 1
                    continue
                spill, keep_waits = (
                    waits[: len(waits) - keep],
                    waits[len(waits) - keep :],
                )
                inst.sync_info = mybir.SyncInfo(on_wait=keep_waits, on_update=updates)
                pos = idx
                for i in range(0, len(spill), 2):
                    _SPILL_SEQ[0] += 1
                    ev = mybir.InstEventSemaphore(
                        name=f"WSPILL-{_SPILL_SEQ[0]}", ins=[], outs=[]
                    )
                    ev.engine = inst.engine
                    ev.sync_info = mybir.SyncInfo(
                        on_wait=spill[i : i + 2], on_update=[]
                    )
                    insts.insert(pos, ev)
                    pos += 1
                    idx += 1
                idx += 1


def _build_nc(nrep=1, mode="full", depth=3, style="c"):
    # mode: "full" | "dma" (skip compute) | "compute" (skip per-rep DMAs)
    nc = bass.Bass()
    # qb[blk, p, c, q]: per-partition 8KB-contiguous block slabs
    qb = nc.declare_dram_parameter(
        "qb", [NBLK, 128, 8, BLK], F16, isOutput=False
    )
    # cpack[p, :]: kt (8*128) | vp (1024) | mt (8) packed along columns
    cpack = nc.declare_dram_parameter(
        "cpack", [128, 8 * 128 + D + N_DEPTHS], F16, isOutput=False
    )
    # small[8, :]: m8w (128) | ones8 (8, row 0) | est (2048, row 0)
    small = nc.declare_dram_parameter(
        "small", [N_DEPTHS, _SMALL_COLS], F16, isOutput=False
    )
    # out[p, tile, d]: per-partition contiguous per block; host transposes
    out = nc.declare_dram_parameter(
        "out", [128, NBLK * NTIL, D], F16, isOutput=True
    )

    with TileContext(nc) as tc:
        with (
            tc.tile_pool(name="const", bufs=1) as cpool,
            tc.tile_pool(name="qts", bufs=2) as qpool,
            tc.tile_pool(name="work", bufs=depth + 1) as wpool,
            tc.tile_pool(name="osb", bufs=3) as opool,
            tc.tile_pool(name="ps_a", bufs=3, space="PSUM") as ps_a,
            tc.tile_pool(name="ps_z", bufs=1, space="PSUM") as ps_z,
            tc.tile_pool(name="ps_o", bufs=2, space="PSUM") as ps_o,
        ):
            q_cache = {}

            def load_q(blk, engine):
                q_b = qpool.tile([128, 8, BLK], F16, tag=f"qt_{blk}")
                engine.dma_start(out=q_b, in_=qb[blk])
                q_cache[blk] = q_b
                return q_b

            # prefetch. Head-latency critical path: the first ST matmul
            # needs only kt chunk 0 + q0 chunk 0, so kt rides first on SP
            # while q0 streams per-chunk on ACT; later chunks arrive under
            # the earlier matmuls.
            cp_t = cpool.tile([128, 8 * 128 + D + N_DEPTHS], F16, tag="cpack")
            nc.sync.dma_start(out=cp_t[:, 0:1024], in_=cpack[:, 0:1024])
            q0 = qpool.tile([128, 8, BLK], F16, tag="qt_0")
            q_cache[0] = q0
            for c in range(8):
                nc.scalar.dma_start(out=q0[:, c, :], in_=qb[0, :, c, :])
            sm_t = cpool.tile([N_DEPTHS, _SMALL_COLS], F16, tag="small")
            nc.sync.dma_start(out=sm_t, in_=small[:])
            nc.sync.dma_start(out=cp_t[:, 1024:], in_=cpack[:, 1024:])
            for b in range(1, NBLK):
                load_q(b, nc.sync if b % 2 == 1 else nc.scalar)

            kt_t = [cp_t[:, c * 128 : (c + 1) * 128] for c in range(8)]
            vp_t = cp_t[:, 1024 : 1024 + D]
            mt_t = cp_t[:, 1024 + D : 1024 + D + N_DEPTHS]
            m8w_t = sm_t[:, _M8W_OFF : _M8W_OFF + 128]
            est8_t = sm_t[:, _EST_OFF : _EST_OFF + ROWS_PER_CORE]

            out_r = out.rearrange("p (b t) d -> p b t d", t=NTIL)

            def stageZ(rep, blk, et):
                """Mask Z-matmul; the e* rank-1 term folds into a DVE add
                (est replicated across the 8 depth rows host-side), keeping
                the rank-1 off the PE queue."""
                qs = blk * BLK
                zt = ps_z.tile([N_DEPTHS, BLK], F32, tag="zt")
                nc.tensor.matmul(zt, lhsT=mt_t, rhs=et, start=True, stop=True)
                ztmp = wpool.tile([N_DEPTHS, BLK], F32, tag="ztmp")
                nc.vector.tensor_add(ztmp, zt, est8_t[:, qs : qs + BLK])
                rt = wpool.tile([N_DEPTHS, BLK], F16, tag="rt")
                with nc.allow_low_precision(reason="attention probs fp16"):
                    nc.vector.reciprocal(rt, ztmp)
                return rt

            def _getq(rep, blk):
                if mode == "compute" or (rep == 0 and blk in q_cache):
                    return q_cache[blk]
                return load_q(blk, nc.scalar if blk % 2 == 0 else nc.sync)

            def stage1p(rep, blka, blkb):
                """ST matmuls for TWO blocks with chunk-interleaved emission:
                consecutive matmuls share the kt stationary, so the dedup
                pass drops half the Ldweights."""
                qa, qvb = _getq(rep, blka), _getq(rep, blkb)
                sta = ps_a.tile([128, BLK], F32, tag="stct")
                stb = ps_a.tile([128, BLK], F32, tag="stct")
                prev_mm = None
                for c in range(8):
                    ma = nc.tensor.matmul(
                        sta, lhsT=kt_t[c], rhs=qa[:, c, :],
                        start=(c == 0), stop=(c == 7), skip_group_check=True,
                    )
                    if prev_mm is not None:
                        # ordering-only chain: keep a/b chunk-interleaved so
                        # consecutive matmuls share the kt stationary and the
                        # Ldweights dedup pass can drop half of them
                        add_dep_helper(ma.ins, prev_mm.ins, sync=False,
                                       reason="st pair interleave")
                    mb = nc.tensor.matmul(
                        stb, lhsT=kt_t[c], rhs=qvb[:, c, :],
                        start=(c == 0), stop=(c == 7), skip_group_check=True,
                    )
                    add_dep_helper(mb.ins, ma.ins, sync=False,
                                   reason="st pair interleave")
                    prev_mm = mb
                eta = wpool.tile([128, BLK], F16, tag="et")
                nc.scalar.activation(
                    eta, sta, mybir.ActivationFunctionType.Exp,
                    scale=float(INV_SQRT_D),
                )
                etb = wpool.tile([128, BLK], F16, tag="et")
                nc.scalar.activation(
                    etb, stb, mybir.ActivationFunctionType.Exp,
                    scale=float(INV_SQRT_D),
                )
                return eta, etb

            def stageC(rt):
                ct = ps_a.tile([128, BLK], F32, tag="stct")
                nc.tensor.matmul(ct, lhsT=m8w_t, rhs=rt, start=True, stop=True)
                return ct

            def stage2(rep, blk, et, ct):
                """A / AV / output drain for a block."""
                at = wpool.tile([128, BLK], F16, tag="at")
                o_blk = opool.tile([128, NTIL, D], F16, tag="osb")
                if style != "a":
                    # one A=E*C per block: fewer DVE ops; depth-3 slack
                    # already covers the coarser dependency
                    nc.vector.tensor_mul(at, et, ct)
                for t in range(NTIL):
                    sl = slice(t * 128, (t + 1) * 128)
                    if style == "a":
                        nc.vector.tensor_mul(at[:, sl], et[:, sl], ct[:, sl])
                    o_ps = ps_o.tile([128, D], F32, tag="ops")
                    nc.tensor.matmul(
                        o_ps[:, 0:512],
                        lhsT=at[:, sl],
                        rhs=vp_t[:, 0:512],
                        start=True,
                        stop=True,
                    )
                    nc.tensor.matmul(
                        o_ps[:, 512:1024],
                        lhsT=at[:, sl],
                        rhs=vp_t[:, 512:1024],
                        start=True,
                        stop=True,
                    )
                    with nc.allow_low_precision(reason="fp16 output"):
                        if style != "a":
                            # full-tile copy on one engine; style c pairs
                            # per half-block so each out-DMA waits on
                            # exactly one producer engine
                            act_gets = (t < NTIL // 2) if style == "c" else (t % 2 == 0)
                            if act_gets:
                                nc.scalar.copy(o_blk[:, t], o_ps)
                            else:
                                nc.vector.tensor_copy(o_blk[:, t], o_ps)
                        else:
                            nc.scalar.copy(o_blk[:, t, 0:640], o_ps[:, 0:640])
                            nc.vector.tensor_copy(
                                o_blk[:, t, 640:1024], o_ps[:, 640:1024]
                            )
                    last = rep == nrep - 1 and blk == NBLK - 1
                    if mode == "compute" and rep != nrep - 1:
                        pass
                    elif last:
                        # final block: per-tile drains on alternating rings
                        # to minimise the tail
                        eng = nc.sync if t % 2 == 0 else nc.gpsimd
                        eng.dma_start(out=out_r[:, blk, t], in_=o_blk[:, t])
                    elif t % 2 == 1:
                        # half-block drains: GPSIMD's SWDGE ring for early
                        # blocks, SP once its input work is done, so
                        # copy-producer waits never stall the input prefetch
                        eng = nc.sync if blk == NBLK - 1 else nc.gpsimd
                        eng.dma_start(
                            out=out_r[:, blk, t - 1 : t + 1],
                            in_=o_blk[:, t - 1 : t + 1],
                        )

            if mode == "dma":
                for i in range(NBLK * nrep):
                    rep, blk = i // NBLK, i % NBLK
                    if rep == 0 and blk in q_cache:
                        q_b = q_cache[blk]
                    else:
                        q_b = load_q(blk, nc.scalar if blk % 2 == 0 else nc.sync)
                    o_blk = opool.tile([128, NTIL, D], F16, tag="osb")
                    for t in range(NTIL):
                        nc.vector.tensor_copy(
                            o_blk[:, t, 0:512], q_b[:, 0, 0:512]
                        )
                        nc.vector.tensor_copy(
                            o_blk[:, t, 512:1024], q_b[:, 1, 0:512]
                        )
                    for half in range(2):
                        eng = nc.sync if blk == NBLK - 1 else nc.gpsimd
                        eng.dma_start(
                            out=out_r[:, blk, half * 2 : half * 2 + 2],
                            in_=o_blk[:, half * 2 : half * 2 + 2],
                        )
            else:
                # pair-level software pipeline: per step the PE order is
                #   Z(prev pair), ST(pair, chunk-interleaved), C+AV(prev)
                npairs = (NBLK // 2) * nrep
                prev = None
                for p in range(npairs):
                    rep = (p * 2) // NBLK
                    ba, bb = (p * 2) % NBLK, (p * 2 + 1) % NBLK
                    if prev is not None:
                        pr, pa, pb, eta_p, etb_p = prev
                        rta = stageZ(pr, pa, eta_p)
                        rtb = stageZ(pr, pb, etb_p)
                    eta, etb = stage1p(rep, ba, bb)
                    if prev is not None:
                        cta = stageC(rta)
                        ctb = stageC(rtb)
                        stage2(pr, pa, eta_p, cta)
                        stage2(pr, pb, etb_p, ctb)
                    prev = (rep, ba, bb, eta, etb)
                pr, pa, pb, eta_p, etb_p = prev
                rta = stageZ(pr, pa, eta_p)
                rtb = stageZ(pr, pb, etb_p)
                cta = stageC(rta)
                ctb = stageC(rtb)
                stage2(pr, pa, eta_p, cta)
                stage2(pr, pb, etb_p, ctb)
    if style == "c":
        _dedupe_ldweights(nc)
    _legalize_sync_commands(nc)
    return nc


def _prepare_in_maps(query, key, value, scale_weights, scale_temperature):
    sw = np.asarray(scale_weights, dtype=np.float64)[:N_DEPTHS]
    temp = float(np.asarray(scale_temperature, dtype=np.float64))
    e = np.exp(sw / temp - np.max(sw / temp))
    w = (e / e.sum()).astype(np.float32)  # [8]

    mt = _MEMBER.T.astype(np.float16)  # [128, 8]
    m8w = (_MEMBER * w[:, None]).astype(np.float16)  # [8, 128]

    in_maps = []
    vstars = []
    for core in range(NCORES):
        b, half = core // 2, core % 2
        rows = slice(half * ROWS_PER_CORE, (half + 1) * ROWS_PER_CORE)
        q = np.ascontiguousarray(query[b, rows])  # [2048, D] f32
        k_u = key[b, _COLS]  # [128, D] f32
        vstar = value[b, 0].astype(np.float32)  # [D]
        vp = (value[b, _COLS] - vstar[None, :]).astype(np.float16)
        s0 = q @ key[b, 0]  # [2048] f32
        est = np.exp(s0 * INV_SQRT_D).astype(np.float16)  # [2048]

        qt = q.T.astype(np.float16)  # [D, 2048]
        # qb[blk, p, c, q] = qt[c*128+p, blk*512+q] -> 8KB/partition slabs
        qb = np.ascontiguousarray(
            qt.reshape(8, 128, NBLK, BLK).transpose(2, 1, 0, 3)
        )
        ktp = np.ascontiguousarray(
            k_u.T.astype(np.float16).reshape(8, 128, 128).transpose(1, 0, 2)
        ).reshape(128, 1024)  # [p, c*128+j]
        cpack = np.concatenate([ktp, vp, mt], axis=1)  # [128, 2056]
        smallt = np.zeros((N_DEPTHS, _SMALL_COLS), dtype=np.float16)
        smallt[:, _M8W_OFF : _M8W_OFF + 128] = m8w
        smallt[0, _ONES_OFF : _ONES_OFF + N_DEPTHS] = 1.0
        smallt[:, _EST_OFF : _EST_OFF + ROWS_PER_CORE] = est[None, :]
        in_maps.append(
            {
                "qb": qb,
                "cpack": np.ascontiguousarray(cpack),
                "small": smallt,
            }
        )
        vstars.append(vstar)
    return in_maps, vstars


def _unshard(results, vstars):
    outp = np.empty((B, L, D), dtype=np.float32)
    for core in range(NCORES):
        b, half = core // 2, core % 2
        rows = slice(half * ROWS_PER_CORE, (half + 1) * ROWS_PER_CORE)
        o = results[core]["out"]  # [128, 16, 1024] fp16
        o = o.transpose(1, 0, 2).reshape(ROWS_PER_CORE, D)
        outp[b, rows] = o.astype(np.float32) + vstars[core][None, :]
    return outp


def _run(query, key, value, t, scale_weights, scale_temperature, trace=False):
    global _NC_CACHE
    query = np.asarray(query, dtype=np.float32)
    key = np.asarray(key, dtype=np.float32)
    value = np.asarray(value, dtype=np.float32)
    assert query.shape == (B, L, D)

    in_maps, vstars = _prepare_in_maps(
        query, key, value, scale_weights, scale_temperature
    )
    if _NC_CACHE is None:
        _NC_CACHE = _build_nc()
    res = run_bass_kernel_spmd(
        _NC_CACHE, in_maps, core_ids=list(range(NCORES)), trace=trace
    )
    return _unshard(res.results, vstars), res


def kernel(query, key, value, t, scale_weights, scale_temperature):
    out, _ = _run(query, key, value, t, scale_weights, scale_temperature, trace=False)
    return out
